# revision 18
# baseline (speedup 1.0000x reference)
"""DeepseekV3 attention on 8 TRN2 NeuronCores.

Sharding: phase 1 token-sharded latent projections (8 blocks of 512 tokens),
AllGather of latents within each 4-core batch group, phase 2 head-sharded
(4 heads per core) attention + partial o_proj; host sums the 4 partials
per batch. All matmuls bf16 with fp32 PSUM accumulation.
"""
import numpy as np
import ml_dtypes

import concourse.bacc as bacc
import concourse.mybir as mybir
import concourse.tile as tile

B, T, HID = 2, 2048, 2048
NH = 16
QLR, KVLR = 1536, 512
DN, DR = 128, 64
DQK, DV = DN + DR, 128
EPS = 1e-6
THETA = 10000.0
SCALE = DQK ** -0.5

NB = 512          # tokens per phase-1 block
HPC = 4           # heads per core in phase 2
AGR = QLR + KVLR + 2 * DR   # 2176 rows in the AllGathered latent buffer

f32 = mybir.dt.float32
bf16 = mybir.dt.bfloat16
Exp = mybir.ActivationFunctionType.Exp
Sqrt = mybir.ActivationFunctionType.Sqrt
Square = mybir.ActivationFunctionType.Square

_BF = ml_dtypes.bfloat16


def _build():
    nc = bacc.Bacc(None, num_devices=8)

    # ---- per-core inputs ----
    xT = nc.declare_dram_parameter("xT", [HID, NB], bf16, isOutput=False)
    wqa = nc.declare_dram_parameter("wqa", [HID, QLR], bf16, isOutput=False)
    wkva = nc.declare_dram_parameter("wkva", [HID, KVLR + 2 * DR], bf16, isOutput=False)
    wqb = nc.declare_dram_parameter("wqb", [QLR, 1024], bf16, isOutput=False)
    wkvk = nc.declare_dram_parameter("wkvk", [KVLR, 512], bf16, isOutput=False)
    wkvv = nc.declare_dram_parameter("wkvv", [KVLR, 512], bf16, isOutput=False)
    wo = nc.declare_dram_parameter("wo", [HPC * DV, HID], bf16, isOutput=False)
    cs = nc.declare_dram_parameter("cs", [128, T], bf16, isOutput=False)  # [c;c;-s;s]
    masks = nc.declare_dram_parameter("masks", [4, 128, 512], bf16, isOutput=False)
    eye2 = nc.declare_dram_parameter("eye2", [128, 64], bf16, isOutput=False)
    out = nc.declare_dram_parameter("out", [T, HID], f32, isOutput=True)

    KVR = KVLR + 2 * DR  # 640 rows: kv_latn | krot | krotswap
    ag_in_kv = nc.dram_tensor("ag_in_kv", [KVR, NB], bf16)
    ag_out_kv = nc.dram_tensor("ag_out_kv", [4, KVR, NB], bf16)
    HB = NB // 2
    ag_in_q0 = nc.dram_tensor("ag_in_q0", [QLR, HB], bf16)
    ag_out_q0 = nc.dram_tensor("ag_out_q0", [4, QLR, HB], bf16)
    ag_in_q1 = nc.dram_tensor("ag_in_q1", [QLR, HB], bf16)
    ag_out_q1 = nc.dram_tensor("ag_out_q1", [4, QLR, HB], bf16)

    with tile.TileContext(nc) as tc:
        # ============ phase 1: latents for own 512-token block ============
        with (
            tc.tile_pool(name="p1", bufs=1) as p1,
            tc.tile_pool(name="p1w", bufs=2) as p1w,
            tc.tile_pool(name="ps1", bufs=3, space="PSUM") as ps1,
            tc.tile_pool(name="ps1acc", bufs=1, space="PSUM") as ps1acc,
        ):
            xt = p1.tile([128, 16, NB], bf16, tag="xt")
            wqat = p1.tile([128, 16, QLR], bf16, tag="wqat")
            wkvat = p1.tile([128, 16, KVLR + 2 * DR], bf16, tag="wkvat")
            for k in range(16):
                nc.sync.dma_start(xt[:, k], xT[128 * k : 128 * (k + 1), :])
                nc.sync.dma_start(wqat[:, k], wqa[128 * k : 128 * (k + 1), :])
            for k in range(16):
                nc.sync.dma_start(wkvat[:, k], wkva[128 * k : 128 * (k + 1), :])
            ones = p1.tile([128, 128], bf16, tag="ones")
            nc.vector.memset(ones[:], 1.0)

            def normalize_and_store(ssq, latt, mt, d, dst_, w):
                mt_ = p1w.tile([128, w], f32, tag="rmst", name="rmst")
                nc.vector.tensor_scalar(
                    mt_[:], ssq[:], 1.0 / d, EPS,
                    mybir.AluOpType.mult, mybir.AluOpType.add,
                )
                rms = p1w.tile([128, w], f32, tag="rms", name="rms")
                nc.scalar.activation(rms[:], mt_[:], Sqrt)
                inv = p1w.tile([128, w], f32, tag="inv", name="inv")
                nc.vector.reciprocal(inv[:], rms[:])
                for m in range(mt):
                    ltn = p1w.tile([128, w], bf16, tag="ltn", name="ltn")
                    nc.vector.tensor_mul(ltn[:], latt[:, m], inv[:])
                    nc.gpsimd.dma_start(dst_[128 * m : 128 * (m + 1), :], ltn[:])

            def q_half(half, agi, ago_):
                c0 = half * HB
                qlat = p1.tile([128, 12, HB], f32, tag="qlat", name="qlat")
                ssq_q = ps1acc.tile([128, HB], f32, tag="ssq_q", name="ssq_q")
                for m in range(12):
                    ps = ps1.tile([128, HB], f32, tag="p1psq", name="p1psq")
                    for k in range(16):
                        nc.tensor.matmul(
                            ps[:], wqat[:, k, 128 * m : 128 * (m + 1)],
                            xt[:, k, c0 : c0 + HB],
                            start=(k == 0), stop=(k == 15),
                        )
                    sq = p1w.tile([128, HB], bf16, tag="sqq", name="sqq")
                    nc.scalar.activation(sq[:], ps[:], Square)
                    nc.vector.tensor_copy(qlat[:, m], ps[:])
                    nc.tensor.matmul(ssq_q[:], ones[:], sq[:],
                                     start=(m == 0), stop=(m == 11))
                normalize_and_store(ssq_q, qlat, 12, QLR, agi, HB)
                nc.gpsimd.collective_compute(
                    "AllGather", mybir.AluOpType.bypass,
                    replica_groups=[[0, 1, 2, 3], [4, 5, 6, 7]],
                    ins=[agi[:]], outs=[ago_[:]],
                )

            q_half(0, ag_in_q0, ag_out_q0)
            # ckv^T first: m 0..3 kv_lat (normed), m 4 = krot+krotswap (raw)
            kvlat = p1.tile([128, 4, NB], f32, tag="kvlat")
            ssq_kv = ps1acc.tile([128, NB], f32, tag="ssq_kv")
            for m in range(5):
                ps = ps1.tile([128, NB], f32, tag="p1ps")
                for k in range(16):
                    nc.tensor.matmul(
                        ps[:], wkvat[:, k, 128 * m : 128 * (m + 1)], xt[:, k],
                        start=(k == 0), stop=(k == 15),
                    )
                if m < 4:
                    sq = p1w.tile([128, NB], bf16, tag="sq")
                    nc.scalar.activation(sq[:], ps[:], Square)
                    nc.vector.tensor_copy(kvlat[:, m], ps[:])
                    nc.tensor.matmul(ssq_kv[:], ones[:], sq[:],
                                     start=(m == 0), stop=(m == 3))
                else:
                    rot = p1w.tile([128, NB], bf16, tag="rot")
                    nc.vector.tensor_copy(rot[:], ps[:])
                    nc.gpsimd.dma_start(ag_in_kv[KVLR : KVR, :], rot[:])
            normalize_and_store(ssq_kv, kvlat, 4, KVLR, ag_in_kv, NB)

            nc.gpsimd.collective_compute(
                "AllGather", mybir.AluOpType.bypass,
                replica_groups=[[0, 1, 2, 3], [4, 5, 6, 7]],
                ins=[ag_in_kv[:]], outs=[ag_out_kv[:]],
            )

            q_half(1, ag_in_q1, ag_out_q1)

        # ============ phase 2: 4 heads, full batch ============
        with tc.tile_pool(name="p2", bufs=1) as p2:
            qTp = p2.tile([128, 4, 4, NB], bf16, tag="qTp")      # pass, per head m
            qrot = [
                p2.tile([64, T], bf16, tag=f"qrot{h}", name=f"qrot{h}")
                for h in range(HPC)
            ]
            krotT = p2.tile([64, T], bf16, tag="krotT")
            kpT = p2.tile([128, 4, 4, NB], bf16, tag="kpT")      # [d, head, r, t]
            vT = p2.tile([128, 16, 512], bf16, tag="vT")         # [t, t-tile, dv]
            attnT = p2.tile([128, 4, 4, NB], bf16, tag="attnT")  # [dv, head, qn, t]
            cst = p2.tile([128, T], bf16, tag="cst")
            nc.sync.dma_start(cst[:], cs[:])
            eyet = p2.tile([128, 64], bf16, tag="eyet")
            nc.sync.dma_start(eyet[:], eye2[:])
            maskt = p2.tile([128, 4, 512], bf16, tag="maskt")
            for m in range(4):
                nc.sync.dma_start(maskt[:, m], masks[m])
            onesb = p2.tile([128, 128], bf16, tag="onesb")
            nc.vector.memset(onesb[:], 1.0)

            psA_cm = tc.tile_pool(name="psA", bufs=2, space="PSUM")
            psA = psA_cm.__enter__()
            psB_cm = tc.tile_pool(name="psB", bufs=1, space="PSUM")
            psB = psB_cm.__enter__()
            # ---- 2a: q^T chunks + rope ----
            wqbt = p2.tile([128, 12, 1024], bf16, tag="wqbt")
            for k in range(12):
                nc.sync.dma_start(wqbt[:, k], wqb[128 * k : 128 * (k + 1), :])
            qrr = p2.tile([128, 4, 4, NB], bf16, tag="qrr")  # rot+swap raw

            def q_chunk(c, agq):
                with tc.tile_pool(name="p2as", bufs=2) as p2as:
                    for r in range(4):
                        qlt = p2as.tile([128, 12, HB], bf16, tag="qlt",
                                        name="qlt")
                        for k in range(12):
                            nc.gpsimd.dma_start(
                                qlt[:, k], agq[r, 128 * k : 128 * (k + 1), :]
                            )
                        for m in range(8):
                            ps = psA.tile([128, HB], f32, tag="q2ps",
                                          name="q2ps")
                            for k in range(12):
                                nc.tensor.matmul(
                                    ps[:], wqbt[:, k, 128 * m : 128 * (m + 1)],
                                    qlt[:, k], start=(k == 0), stop=(k == 11),
                                )
                            dst = qTp[:, m, r] if m < 4 else qrr[:, m - 4, r]
                            nc.vector.tensor_copy(
                                dst[:, c * HB : (c + 1) * HB], ps[:]
                            )

            def q_rope():
                with tc.tile_pool(name="p2ar", bufs=2) as p2ar:
                    for h in range(HPC):
                        for r in range(4):
                            tt = p2ar.tile([128, NB], bf16, tag="ropet",
                                           name="ropet")
                            nc.vector.tensor_mul(
                                tt[:], qrr[:, h, r],
                                cst[:, 512 * r : 512 * (r + 1)],
                            )
                            pr = psB.tile([64, NB], f32, tag="ropeps",
                                          name="ropeps")
                            nc.tensor.matmul(pr[:], eyet[:], tt[:],
                                             start=True, stop=True)
                            nc.vector.tensor_copy(
                                qrot[h][:, 512 * r : 512 * (r + 1)], pr[:]
                            )

            q_chunk(0, ag_out_q0)
            # ---- 2b/2c: k_pass^T, V, k_rot rope ----
            with tc.tile_pool(name="p2b", bufs=1) as p2b:
                wkkt = p2b.tile([128, 4, 512], bf16, tag="wkkt")
                wkvt = p2b.tile([128, 4, 512], bf16, tag="wkvt")
                for k in range(4):
                    nc.sync.dma_start(wkkt[:, k], wkvk[128 * k : 128 * (k + 1), :])
                    nc.sync.dma_start(wkvt[:, k], wkvv[128 * k : 128 * (k + 1), :])
                kvl = p2b.tile([128, 4, 4, NB], bf16, tag="kvl")  # [r_lat, k, r, t]
                krr = p2b.tile([128, 4, NB], bf16, tag="krr")
                for r in range(4):
                    for k in range(4):
                        nc.gpsimd.dma_start(
                            kvl[:, k, r],
                            ag_out_kv[r, 128 * k : 128 * (k + 1), :],
                        )
                    nc.gpsimd.dma_start(krr[:, r], ag_out_kv[r, KVLR : KVR, :])
                # k_pass^T
                for m in range(4):
                    for r in range(4):
                        ps = psA.tile([128, NB], f32, tag="k2ps")
                        for k in range(4):
                            nc.tensor.matmul(
                                ps[:], wkkt[:, k, 128 * m : 128 * (m + 1)],
                                kvl[:, k, r], start=(k == 0), stop=(k == 3),
                            )
                        nc.vector.tensor_copy(kpT[:, m, r], ps[:])
                # V token-major
                for r in range(4):
                    for s in range(4):
                        ps = psA.tile([128, 512], f32, tag="v2ps")
                        for k in range(4):
                            nc.tensor.matmul(
                                ps[:], kvl[:, k, r, 128 * s : 128 * (s + 1)],
                                wkvt[:, k], start=(k == 0), stop=(k == 3),
                            )
                        nc.vector.tensor_copy(vT[:, 4 * r + s], ps[:])
                # k_rot rope
                for r in range(4):
                    tt = p2b.tile([128, NB], bf16, tag="kropet")
                    nc.vector.tensor_mul(
                        tt[:], krr[:, r], cst[:, 512 * r : 512 * (r + 1)]
                    )
                    pr = psB.tile([64, NB], f32, tag="kropeps")
                    nc.tensor.matmul(pr[:], eyet[:], tt[:], start=True, stop=True)
                    nc.vector.tensor_copy(krotT[:, 512 * r : 512 * (r + 1)], pr[:])

            q_chunk(1, ag_out_q1)
            q_rope()
            psB_cm.__exit__(None, None, None)
            psA_cm.__exit__(None, None, None)

            wot = p2.tile([128, 4, HID], bf16, tag="wot")
            for k in range(4):
                nc.sync.dma_start(wot[:, k], wo[128 * k : 128 * (k + 1), :])

            # ---- 2d: attention ----
            with (
                tc.tile_pool(name="p2d", bufs=6) as p2d,
                tc.tile_pool(name="p2dn", bufs=2) as p2dn,
                tc.tile_pool(name="ps2s", bufs=4, space="PSUM") as ps2s,
                tc.tile_pool(name="ps2acc", bufs=2, space="PSUM") as ps2acc,
            ):
                for h in range(HPC):
                    for qn in range(4):
                        nkt = 4 * qn + 4
                        aps = ps2acc.tile([128, NB], f32, tag="attn_ps")
                        sps = ps2acc.tile([128, NB], f32, tag="sum_ps")
                        eac = p2dn.tile([128, NB], bf16, tag="eacc", name="eacc")
                        prev = None

                        def emit_pv(pe, pk, off, last):
                            nc.tensor.matmul(
                                aps[:, off:NB], vT[:, pk, 128 * h : 128 * (h + 1)],
                                pe[:, off:NB], start=(pk == 0), stop=last,
                            )
                            if pk == 0:
                                nc.vector.tensor_copy(eac[:], pe[:])
                            else:
                                nc.vector.tensor_add(
                                    eac[:, off:NB], eac[:, off:NB], pe[:, off:NB]
                                )
                            if last:
                                nc.tensor.matmul(
                                    sps[:], onesb[:], eac[:],
                                    start=True, stop=True,
                                )

                        for kt in range(nkt):
                            m = kt - 4 * qn
                            off = 128 * m if m > 0 else 0
                            scp = ps2s.tile([128, NB], f32, tag="scps")
                            r, sl = kt // 4, 128 * (kt % 4)
                            nc.tensor.matmul(
                                scp[:, off:NB], kpT[:, h, r, sl : sl + 128],
                                qTp[:, h, qn, off:NB], start=True, stop=False,
                            )
                            nc.tensor.matmul(
                                scp[:, off:NB], krotT[:, 128 * kt : 128 * kt + 128],
                                qrot[h][:, 512 * qn + off : 512 * (qn + 1)],
                                start=False, stop=True,
                            )
                            et = p2d.tile([128, NB], bf16, tag="expT")
                            nc.scalar.activation(
                                et[:, off:NB], scp[:, off:NB], Exp, scale=SCALE
                            )
                            if m >= 0:
                                nc.vector.tensor_mul(
                                    et[:, off:NB], et[:, off:NB],
                                    maskt[:, m, off:NB],
                                )
                            if prev is not None:
                                emit_pv(prev[0], prev[1], prev[2], False)
                            prev = (et, kt, off)
                        emit_pv(prev[0], prev[1], prev[2], True)
                        rec = p2dn.tile([128, NB], f32, tag="rec")
                        nc.vector.reciprocal(rec[:], sps[:])
                        nc.vector.tensor_mul(attnT[:, h, qn], aps[:], rec[:])

            # ---- 2e: o_proj partial (token-major out) ----
            with (
                tc.tile_pool(name="p2eo", bufs=3) as p2eo,
                tc.tile_pool(name="ps2o", bufs=3, space="PSUM") as ps2o,
            ):
                for tm in range(16):
                    qn, s = tm // 4, 128 * (tm % 4)
                    for n in range(4):
                        ps = ps2o.tile([128, 512], f32, tag="ops")
                        for k in range(4):
                            nc.tensor.matmul(
                                ps[:], attnT[:, k, qn, s : s + 128],
                                wot[:, k, 512 * n : 512 * (n + 1)],
                                start=(k == 0), stop=(k == 3),
                            )
                        ot = p2eo.tile([128, 512], f32, tag="oT")
                        nc.vector.tensor_copy(ot[:], ps[:])
                        nc.sync.dma_start(
                            out[128 * tm : 128 * (tm + 1),
                                512 * n : 512 * (n + 1)], ot[:],
                        )

    nc.finalize()
    return nc


_NC = None


def _get_nc():
    global _NC
    if _NC is None:
        _NC = _build()
    return _NC


def _prep_inputs(x, attention_mask, positions, wqa, qa_scale, wqb, wkva,
                 kva_scale, wkvb, wo):
    x = np.asarray(x, np.float32)
    positions = np.asarray(positions)
    wqa = np.asarray(wqa, np.float32)
    wqb = np.asarray(wqb, np.float32) * np.asarray(qa_scale, np.float32)[:, None]
    wkva = np.asarray(wkva, np.float32)
    wkvb = np.asarray(wkvb, np.float32) * np.asarray(kva_scale, np.float32)[:, None]
    wo = np.asarray(wo, np.float32)

    # wkva augmented with swapped-rot columns
    kr = wkva[:, KVLR:]
    wkva_aug = np.concatenate(
        [wkva[:, :KVLR], kr, kr[:, DR // 2 :], kr[:, : DR // 2]], axis=1
    ).astype(_BF)

    # masks: mask[m][r, c] = c >= 128*m + r
    rr = np.arange(128)[:, None]
    cc = np.arange(512)[None, :]
    masks = np.stack([(cc >= 128 * m + rr) for m in range(4)]).astype(_BF)

    eye2 = np.concatenate([np.eye(64), np.eye(64)], axis=0).astype(_BF)

    # per-batch cos/sin stack [c; c; -s; s]
    inv_freq = 1.0 / (THETA ** (np.arange(0, DR, 2, dtype=np.float32) / DR))
    cs_b = []
    for b in range(B):
        ang = positions[b].astype(np.float32)[None, :] * inv_freq[:, None]
        c, s = np.cos(ang), np.sin(ang)
        cs_b.append(np.concatenate([c, c, -s, s], axis=0).astype(_BF))

    wqa_bf = wqa.astype(_BF)
    in_maps = []
    for core in range(8):
        b, j = core // 4, core % 4
        hs = [4 * (core % 4) + i for i in range(HPC)]
        # wqb per head-group: [pass x4 | (rot,swap) per head x4]
        cols = [wqb[:, h * DQK : h * DQK + DN] for h in hs]
        for h in hs:
            rot = wqb[:, h * DQK + DN : (h + 1) * DQK]
            cols.append(rot)
            cols.append(np.concatenate(
                [rot[:, DR // 2 :], rot[:, : DR // 2]], axis=1))
        wqb_hg = np.concatenate(cols, axis=1).astype(_BF)
        wkvk_hg = np.concatenate(
            [wkvb[:, h * (DN + DV) : h * (DN + DV) + DN] for h in hs], axis=1
        ).astype(_BF)
        wkvv_hg = np.concatenate(
            [wkvb[:, h * (DN + DV) + DN : (h + 1) * (DN + DV)] for h in hs], axis=1
        ).astype(_BF)
        wo_hg = wo[hs[0] * DV : (hs[-1] + 1) * DV, :].astype(_BF)
        xTb = np.ascontiguousarray(
            x[b, NB * j : NB * (j + 1), :].T).astype(_BF)
        in_maps.append({
            "xT": xTb,
            "wqa": wqa_bf,
            "wkva": wkva_aug,
            "wqb": wqb_hg,
            "wkvk": wkvk_hg,
            "wkvv": wkvv_hg,
            "wo": wo_hg,
            "cs": cs_b[b],
            "masks": masks,
            "eye2": eye2,
        })
    return in_maps


def _run(inputs, trace=False, trace_kwargs=None):
    from concourse.bass_utils import run_bass_kernel_spmd

    nc = _get_nc()
    in_maps = _prep_inputs(**inputs)
    res = run_bass_kernel_spmd(
        nc, in_maps, list(range(8)), trace=trace,
        trace_kwargs=trace_kwargs or {},
    )
    outs = np.zeros((B, T, HID), np.float32)
    for core in range(8):
        outs[core // 4] += res.results[core]["out"]
    return outs, res


def kernel(**inputs) -> np.ndarray:
    out, _ = _run(inputs)
    return out


# revision 19
# speedup vs baseline: 1.1433x; 1.1433x over previous
"""DeepseekV3 attention on 8 TRN2 NeuronCores.

Sharding: phase 1 token-sharded latent projections (8 blocks of 512 tokens),
AllGather of latents within each 4-core batch group, phase 2 head-sharded
(4 heads per core) attention + partial o_proj; host sums the 4 partials
per batch. All matmuls bf16 with fp32 PSUM accumulation.
"""
import numpy as np
import ml_dtypes

import concourse.bacc as bacc
import concourse.mybir as mybir
import concourse.tile as tile

B, T, HID = 2, 2048, 2048
NH = 16
QLR, KVLR = 1536, 512
DN, DR = 128, 64
DQK, DV = DN + DR, 128
EPS = 1e-6
THETA = 10000.0
SCALE = DQK ** -0.5

NB = 512          # tokens per phase-1 block
HPC = 4           # heads per core in phase 2
AGR = QLR + KVLR + 2 * DR   # 2176 rows in the AllGathered latent buffer

f32 = mybir.dt.float32
bf16 = mybir.dt.bfloat16
Exp = mybir.ActivationFunctionType.Exp
Sqrt = mybir.ActivationFunctionType.Sqrt
Square = mybir.ActivationFunctionType.Square

_BF = ml_dtypes.bfloat16


def _build():
    nc = bacc.Bacc(None, num_devices=8)

    # ---- per-core inputs ----
    xT = nc.declare_dram_parameter("xT", [HID, NB], bf16, isOutput=False)
    wqa = nc.declare_dram_parameter("wqa", [HID, QLR], bf16, isOutput=False)
    wkva = nc.declare_dram_parameter("wkva", [HID, KVLR + 2 * DR], bf16, isOutput=False)
    wqb = nc.declare_dram_parameter("wqb", [QLR, 1024], bf16, isOutput=False)
    wkvk = nc.declare_dram_parameter("wkvk", [KVLR, 512], bf16, isOutput=False)
    wkvv = nc.declare_dram_parameter("wkvv", [KVLR, 512], bf16, isOutput=False)
    wo = nc.declare_dram_parameter("wo", [HPC * DV, HID], bf16, isOutput=False)
    cs = nc.declare_dram_parameter("cs", [128, T], bf16, isOutput=False)  # [c;c;-s;s]
    masks = nc.declare_dram_parameter("masks", [4, 128, 512], bf16, isOutput=False)
    eye2 = nc.declare_dram_parameter("eye2", [128, 64], bf16, isOutput=False)
    out = nc.declare_dram_parameter("out", [T, HID], f32, isOutput=True)

    KVR = KVLR + 2 * DR  # 640 rows: kv_latn | krot | krotswap
    ag_in_kv = nc.dram_tensor("ag_in_kv", [KVR, NB], bf16)
    ag_out_kv = nc.dram_tensor("ag_out_kv", [4, KVR, NB], bf16)
    HB = NB // 2
    ag_in_q0 = nc.dram_tensor("ag_in_q0", [QLR, HB], bf16)
    ag_out_q0 = nc.dram_tensor("ag_out_q0", [4, QLR, HB], bf16)
    ag_in_q1 = nc.dram_tensor("ag_in_q1", [QLR, HB], bf16)
    ag_out_q1 = nc.dram_tensor("ag_out_q1", [4, QLR, HB], bf16)

    with tile.TileContext(nc) as tc:
        # ============ phase 1: latents for own 512-token block ============
        with (
            tc.tile_pool(name="p1", bufs=1) as p1,
            tc.tile_pool(name="p1w", bufs=2) as p1w,
            tc.tile_pool(name="ps1", bufs=3, space="PSUM") as ps1,
            tc.tile_pool(name="ps1acc", bufs=1, space="PSUM") as ps1acc,
        ):
            xt = p1.tile([128, 16, NB], bf16, tag="xt")
            wqat = p1.tile([128, 16, QLR], bf16, tag="wqat")
            wkvat = p1.tile([128, 16, KVLR + 2 * DR], bf16, tag="wkvat")
            for k in range(16):
                nc.sync.dma_start(xt[:, k], xT[128 * k : 128 * (k + 1), :])
                nc.sync.dma_start(wkvat[:, k], wkva[128 * k : 128 * (k + 1), :])
            for k in range(16):
                nc.sync.dma_start(wqat[:, k], wqa[128 * k : 128 * (k + 1), :])
            ones = p1.tile([128, 128], bf16, tag="ones")
            nc.vector.memset(ones[:], 1.0)

            def normalize_and_store(ssq, latt, mt, d, dst_, w):
                mt_ = p1w.tile([128, w], f32, tag="rmst", name="rmst")
                nc.vector.tensor_scalar(
                    mt_[:], ssq[:], 1.0 / d, EPS,
                    mybir.AluOpType.mult, mybir.AluOpType.add,
                )
                rms = p1w.tile([128, w], f32, tag="rms", name="rms")
                nc.scalar.activation(rms[:], mt_[:], Sqrt)
                inv = p1w.tile([128, w], f32, tag="inv", name="inv")
                nc.vector.reciprocal(inv[:], rms[:])
                for m in range(mt):
                    ltn = p1w.tile([128, w], bf16, tag="ltn", name="ltn")
                    nc.vector.tensor_mul(ltn[:], latt[:, m], inv[:])
                    nc.gpsimd.dma_start(dst_[128 * m : 128 * (m + 1), :], ltn[:])

            # ckv^T first: m 0..3 kv_lat (normed), m 4 = krot+krotswap (raw)
            kvlat = p1.tile([128, 4, NB], f32, tag="kvlat")
            ssq_kv = ps1acc.tile([128, NB], f32, tag="ssq_kv")
            for m in range(5):
                ps = ps1.tile([128, NB], f32, tag="p1ps")
                for k in range(16):
                    nc.tensor.matmul(
                        ps[:], wkvat[:, k, 128 * m : 128 * (m + 1)], xt[:, k],
                        start=(k == 0), stop=(k == 15),
                    )
                if m < 4:
                    sq = p1w.tile([128, NB], bf16, tag="sq")
                    nc.scalar.activation(sq[:], ps[:], Square)
                    nc.vector.tensor_copy(kvlat[:, m], ps[:])
                    nc.tensor.matmul(ssq_kv[:], ones[:], sq[:],
                                     start=(m == 0), stop=(m == 3))
                else:
                    rot = p1w.tile([128, NB], bf16, tag="rot")
                    nc.vector.tensor_copy(rot[:], ps[:])
                    nc.gpsimd.dma_start(ag_in_kv[KVLR : KVR, :], rot[:])
            normalize_and_store(ssq_kv, kvlat, 4, KVLR, ag_in_kv, NB)

            nc.gpsimd.collective_compute(
                "AllGather", mybir.AluOpType.bypass,
                replica_groups=[[0, 1, 2, 3], [4, 5, 6, 7]],
                ins=[ag_in_kv[:]], outs=[ag_out_kv[:]],
            )

            # q_lat^T in two column halves so AG_q0 can start early
            for half, (agi, ago_) in enumerate(
                ((ag_in_q0, ag_out_q0), (ag_in_q1, ag_out_q1))
            ):
                c0 = half * HB
                qlat = p1.tile([128, 12, HB], f32, tag="qlat", name="qlat")
                ssq_q = ps1acc.tile([128, HB], f32, tag="ssq_q", name="ssq_q")
                for m in range(12):
                    ps = ps1.tile([128, HB], f32, tag="p1psq", name="p1psq")
                    for k in range(16):
                        nc.tensor.matmul(
                            ps[:], wqat[:, k, 128 * m : 128 * (m + 1)],
                            xt[:, k, c0 : c0 + HB],
                            start=(k == 0), stop=(k == 15),
                        )
                    sq = p1w.tile([128, HB], bf16, tag="sqq", name="sqq")
                    nc.scalar.activation(sq[:], ps[:], Square)
                    nc.vector.tensor_copy(qlat[:, m], ps[:])
                    nc.tensor.matmul(ssq_q[:], ones[:], sq[:],
                                     start=(m == 0), stop=(m == 11))
                normalize_and_store(ssq_q, qlat, 12, QLR, agi, HB)
                nc.gpsimd.collective_compute(
                    "AllGather", mybir.AluOpType.bypass,
                    replica_groups=[[0, 1, 2, 3], [4, 5, 6, 7]],
                    ins=[agi[:]], outs=[ago_[:]],
                )

        # ============ phase 2: 4 heads, full batch ============
        with tc.tile_pool(name="p2", bufs=1) as p2:
            qTp = p2.tile([128, 4, 4, NB], bf16, tag="qTp")      # pass, per head m
            qrot = [
                p2.tile([64, T], bf16, tag=f"qrot{h}", name=f"qrot{h}")
                for h in range(HPC)
            ]
            krotT = p2.tile([64, T], bf16, tag="krotT")
            kpT = p2.tile([128, 4, 4, NB], bf16, tag="kpT")      # [d, head, r, t]
            vT = p2.tile([128, 16, 512], bf16, tag="vT")         # [t, t-tile, dv]
            attnT = p2.tile([128, 4, 4, NB], bf16, tag="attnT")  # [dv, head, qn, t]
            cst = p2.tile([128, T], bf16, tag="cst")
            nc.sync.dma_start(cst[:], cs[:])
            eyet = p2.tile([128, 64], bf16, tag="eyet")
            nc.sync.dma_start(eyet[:], eye2[:])
            maskt = p2.tile([128, 4, 512], bf16, tag="maskt")
            for m in range(4):
                nc.sync.dma_start(maskt[:, m], masks[m])
            onesb = p2.tile([128, 128], bf16, tag="onesb")
            nc.vector.memset(onesb[:], 1.0)

            # ---- 2b/2c: k_pass^T, V, k_rot rope ----
            with (
                tc.tile_pool(name="p2b", bufs=1) as p2b,
                tc.tile_pool(name="ps2", bufs=2, space="PSUM") as ps2,
            ):
                wkkt = p2b.tile([128, 4, 512], bf16, tag="wkkt")
                wkvt = p2b.tile([128, 4, 512], bf16, tag="wkvt")
                for k in range(4):
                    nc.sync.dma_start(wkkt[:, k], wkvk[128 * k : 128 * (k + 1), :])
                    nc.sync.dma_start(wkvt[:, k], wkvv[128 * k : 128 * (k + 1), :])
                kvl = p2b.tile([128, 4, 4, NB], bf16, tag="kvl")  # [r_lat, k, r, t]
                krr = p2b.tile([128, 4, NB], bf16, tag="krr")
                for r in range(4):
                    for k in range(4):
                        nc.gpsimd.dma_start(
                            kvl[:, k, r],
                            ag_out_kv[r, 128 * k : 128 * (k + 1), :],
                        )
                    nc.gpsimd.dma_start(krr[:, r], ag_out_kv[r, KVLR : KVR, :])
                # k_pass^T
                for m in range(4):
                    for r in range(4):
                        ps = ps2.tile([128, NB], f32, tag="k2ps")
                        for k in range(4):
                            nc.tensor.matmul(
                                ps[:], wkkt[:, k, 128 * m : 128 * (m + 1)],
                                kvl[:, k, r], start=(k == 0), stop=(k == 3),
                            )
                        nc.vector.tensor_copy(kpT[:, m, r], ps[:])
                # V token-major
                for r in range(4):
                    for s in range(4):
                        ps = ps2.tile([128, 512], f32, tag="v2ps")
                        for k in range(4):
                            nc.tensor.matmul(
                                ps[:], kvl[:, k, r, 128 * s : 128 * (s + 1)],
                                wkvt[:, k], start=(k == 0), stop=(k == 3),
                            )
                        nc.vector.tensor_copy(vT[:, 4 * r + s], ps[:])
                # k_rot rope
                for r in range(4):
                    tt = p2b.tile([128, NB], bf16, tag="kropet")
                    nc.vector.tensor_mul(
                        tt[:], krr[:, r], cst[:, 512 * r : 512 * (r + 1)]
                    )
                    pr = ps2.tile([64, NB], f32, tag="kropeps")
                    nc.tensor.matmul(pr[:], eyet[:], tt[:], start=True, stop=True)
                    nc.vector.tensor_copy(krotT[:, 512 * r : 512 * (r + 1)], pr[:])

            # ---- 2a: q^T (8 m-tiles x 4 blocks, K=12) + rope ----
            with (
                tc.tile_pool(name="p2a", bufs=1) as p2a,
                tc.tile_pool(name="p2as", bufs=2) as p2as,
                tc.tile_pool(name="ps2", bufs=2, space="PSUM") as ps2,
            ):
                wqbt = p2a.tile([128, 12, 1024], bf16, tag="wqbt")
                for k in range(12):
                    nc.sync.dma_start(wqbt[:, k], wqb[128 * k : 128 * (k + 1), :])
                qrr = p2a.tile([128, 4, 4, NB], bf16, tag="qrr")  # rot+swap raw
                for c, agq in ((0, ag_out_q0), (1, ag_out_q1)):
                    for r in range(4):
                        qlt = p2as.tile([128, 12, HB], bf16, tag="qlt")
                        for k in range(12):
                            nc.gpsimd.dma_start(
                                qlt[:, k], agq[r, 128 * k : 128 * (k + 1), :]
                            )
                        for m in range(8):
                            ps = ps2.tile([128, HB], f32, tag="q2ps")
                            for k in range(12):
                                nc.tensor.matmul(
                                    ps[:], wqbt[:, k, 128 * m : 128 * (m + 1)],
                                    qlt[:, k], start=(k == 0), stop=(k == 11),
                                )
                            dst = qTp[:, m, r] if m < 4 else qrr[:, m - 4, r]
                            nc.vector.tensor_copy(
                                dst[:, c * HB : (c + 1) * HB], ps[:]
                            )
                # rope q: per head, CS-mul then identity-stack matmul
                for h in range(HPC):
                    for r in range(4):
                        tt = p2as.tile([128, NB], bf16, tag="ropet")
                        nc.vector.tensor_mul(
                            tt[:], qrr[:, h, r], cst[:, 512 * r : 512 * (r + 1)]
                        )
                        pr = ps2.tile([64, NB], f32, tag="ropeps")
                        nc.tensor.matmul(pr[:], eyet[:], tt[:], start=True, stop=True)
                        nc.vector.tensor_copy(
                            qrot[h][:, 512 * r : 512 * (r + 1)], pr[:]
                        )

            wot = p2.tile([128, 4, HID], bf16, tag="wot")
            for k in range(4):
                nc.sync.dma_start(wot[:, k], wo[128 * k : 128 * (k + 1), :])

            # ---- 2d: attention ----
            with (
                tc.tile_pool(name="p2d", bufs=6) as p2d,
                tc.tile_pool(name="p2dn", bufs=2) as p2dn,
                tc.tile_pool(name="ps2s", bufs=4, space="PSUM") as ps2s,
                tc.tile_pool(name="ps2acc", bufs=2, space="PSUM") as ps2acc,
            ):
                for h in range(HPC):
                    for qn in range(4):
                        nkt = 4 * qn + 4
                        aps = ps2acc.tile([128, NB], f32, tag="attn_ps")
                        sps = ps2acc.tile([128, NB], f32, tag="sum_ps")
                        eac = p2dn.tile([128, NB], bf16, tag="eacc", name="eacc")
                        prev = None

                        def emit_pv(pe, pk, off, last):
                            nc.tensor.matmul(
                                aps[:, off:NB], vT[:, pk, 128 * h : 128 * (h + 1)],
                                pe[:, off:NB], start=(pk == 0), stop=last,
                            )
                            if pk == 0:
                                nc.vector.tensor_copy(eac[:], pe[:])
                            else:
                                nc.vector.tensor_add(
                                    eac[:, off:NB], eac[:, off:NB], pe[:, off:NB]
                                )
                            if last:
                                nc.tensor.matmul(
                                    sps[:], onesb[:], eac[:],
                                    start=True, stop=True,
                                )

                        for kt in range(nkt):
                            m = kt - 4 * qn
                            off = 128 * m if m > 0 else 0
                            scp = ps2s.tile([128, NB], f32, tag="scps")
                            r, sl = kt // 4, 128 * (kt % 4)
                            nc.tensor.matmul(
                                scp[:, off:NB], kpT[:, h, r, sl : sl + 128],
                                qTp[:, h, qn, off:NB], start=True, stop=False,
                            )
                            nc.tensor.matmul(
                                scp[:, off:NB], krotT[:, 128 * kt : 128 * kt + 128],
                                qrot[h][:, 512 * qn + off : 512 * (qn + 1)],
                                start=False, stop=True,
                            )
                            et = p2d.tile([128, NB], bf16, tag="expT")
                            nc.scalar.activation(
                                et[:, off:NB], scp[:, off:NB], Exp, scale=SCALE
                            )
                            if m >= 0:
                                nc.vector.tensor_mul(
                                    et[:, off:NB], et[:, off:NB],
                                    maskt[:, m, off:NB],
                                )
                            if prev is not None:
                                emit_pv(prev[0], prev[1], prev[2], False)
                            prev = (et, kt, off)
                        emit_pv(prev[0], prev[1], prev[2], True)
                        rec = p2dn.tile([128, NB], f32, tag="rec")
                        nc.vector.reciprocal(rec[:], sps[:])
                        nc.vector.tensor_mul(attnT[:, h, qn], aps[:], rec[:])

            # ---- 2e: o_proj partial (token-major out) ----
            with (
                tc.tile_pool(name="p2eo", bufs=3) as p2eo,
                tc.tile_pool(name="ps2o", bufs=3, space="PSUM") as ps2o,
            ):
                for tm in range(16):
                    qn, s = tm // 4, 128 * (tm % 4)
                    for n in range(4):
                        ps = ps2o.tile([128, 512], f32, tag="ops")
                        for k in range(4):
                            nc.tensor.matmul(
                                ps[:], attnT[:, k, qn, s : s + 128],
                                wot[:, k, 512 * n : 512 * (n + 1)],
                                start=(k == 0), stop=(k == 3),
                            )
                        ot = p2eo.tile([128, 512], f32, tag="oT")
                        nc.vector.tensor_copy(ot[:], ps[:])
                        nc.sync.dma_start(
                            out[128 * tm : 128 * (tm + 1),
                                512 * n : 512 * (n + 1)], ot[:],
                        )

    nc.finalize()
    return nc


_NC = None


def _get_nc():
    global _NC
    if _NC is None:
        _NC = _build()
    return _NC


def _prep_inputs(x, attention_mask, positions, wqa, qa_scale, wqb, wkva,
                 kva_scale, wkvb, wo):
    x = np.asarray(x, np.float32)
    positions = np.asarray(positions)
    wqa = np.asarray(wqa, np.float32)
    wqb = np.asarray(wqb, np.float32) * np.asarray(qa_scale, np.float32)[:, None]
    wkva = np.asarray(wkva, np.float32)
    wkvb = np.asarray(wkvb, np.float32) * np.asarray(kva_scale, np.float32)[:, None]
    wo = np.asarray(wo, np.float32)

    # wkva augmented with swapped-rot columns
    kr = wkva[:, KVLR:]
    wkva_aug = np.concatenate(
        [wkva[:, :KVLR], kr, kr[:, DR // 2 :], kr[:, : DR // 2]], axis=1
    ).astype(_BF)

    # masks: mask[m][r, c] = c >= 128*m + r
    rr = np.arange(128)[:, None]
    cc = np.arange(512)[None, :]
    masks = np.stack([(cc >= 128 * m + rr) for m in range(4)]).astype(_BF)

    eye2 = np.concatenate([np.eye(64), np.eye(64)], axis=0).astype(_BF)

    # per-batch cos/sin stack [c; c; -s; s]
    inv_freq = 1.0 / (THETA ** (np.arange(0, DR, 2, dtype=np.float32) / DR))
    cs_b = []
    for b in range(B):
        ang = positions[b].astype(np.float32)[None, :] * inv_freq[:, None]
        c, s = np.cos(ang), np.sin(ang)
        cs_b.append(np.concatenate([c, c, -s, s], axis=0).astype(_BF))

    wqa_bf = wqa.astype(_BF)
    in_maps = []
    for core in range(8):
        b, j = core // 4, core % 4
        hs = [4 * (core % 4) + i for i in range(HPC)]
        # wqb per head-group: [pass x4 | (rot,swap) per head x4]
        cols = [wqb[:, h * DQK : h * DQK + DN] for h in hs]
        for h in hs:
            rot = wqb[:, h * DQK + DN : (h + 1) * DQK]
            cols.append(rot)
            cols.append(np.concatenate(
                [rot[:, DR // 2 :], rot[:, : DR // 2]], axis=1))
        wqb_hg = np.concatenate(cols, axis=1).astype(_BF)
        wkvk_hg = np.concatenate(
            [wkvb[:, h * (DN + DV) : h * (DN + DV) + DN] for h in hs], axis=1
        ).astype(_BF)
        wkvv_hg = np.concatenate(
            [wkvb[:, h * (DN + DV) + DN : (h + 1) * (DN + DV)] for h in hs], axis=1
        ).astype(_BF)
        wo_hg = wo[hs[0] * DV : (hs[-1] + 1) * DV, :].astype(_BF)
        xTb = np.ascontiguousarray(
            x[b, NB * j : NB * (j + 1), :].T).astype(_BF)
        in_maps.append({
            "xT": xTb,
            "wqa": wqa_bf,
            "wkva": wkva_aug,
            "wqb": wqb_hg,
            "wkvk": wkvk_hg,
            "wkvv": wkvv_hg,
            "wo": wo_hg,
            "cs": cs_b[b],
            "masks": masks,
            "eye2": eye2,
        })
    return in_maps


def _run(inputs, trace=False, trace_kwargs=None):
    from concourse.bass_utils import run_bass_kernel_spmd

    nc = _get_nc()
    in_maps = _prep_inputs(**inputs)
    res = run_bass_kernel_spmd(
        nc, in_maps, list(range(8)), trace=trace,
        trace_kwargs=trace_kwargs or {},
    )
    outs = np.zeros((B, T, HID), np.float32)
    for core in range(8):
        outs[core // 4] += res.results[core]["out"]
    return outs, res


def kernel(**inputs) -> np.ndarray:
    out, _ = _run(inputs)
    return out


# revision 20
# speedup vs baseline: 1.3299x; 1.1631x over previous
"""DeepseekV3 attention on 8 TRN2 NeuronCores.

Sharding: phase 1 token-sharded latent projections (8 blocks of 512 tokens),
AllGather of latents within each 4-core batch group, phase 2 head-sharded
(4 heads per core) attention + partial o_proj; host sums the 4 partials
per batch. All matmuls bf16 with fp32 PSUM accumulation.
"""
import numpy as np
import ml_dtypes

import concourse.bacc as bacc
import concourse.mybir as mybir
import concourse.tile as tile

B, T, HID = 2, 2048, 2048
NH = 16
QLR, KVLR = 1536, 512
DN, DR = 128, 64
DQK, DV = DN + DR, 128
EPS = 1e-6
THETA = 10000.0
SCALE = DQK ** -0.5

NB = 512          # tokens per phase-1 block
HPC = 4           # heads per core in phase 2
AGR = QLR + KVLR + 2 * DR   # 2176 rows in the AllGathered latent buffer

f32 = mybir.dt.float32
bf16 = mybir.dt.bfloat16
Exp = mybir.ActivationFunctionType.Exp
Sqrt = mybir.ActivationFunctionType.Sqrt
Square = mybir.ActivationFunctionType.Square

_BF = ml_dtypes.bfloat16


def _build():
    nc = bacc.Bacc(None, num_devices=8)

    # ---- per-core inputs ----
    xT = nc.declare_dram_parameter("xT", [HID, NB], bf16, isOutput=False)
    wqa = nc.declare_dram_parameter("wqa", [HID, QLR], bf16, isOutput=False)
    wkva = nc.declare_dram_parameter("wkva", [HID, KVLR + 2 * DR], bf16, isOutput=False)
    wqb = nc.declare_dram_parameter("wqb", [QLR, 768], bf16, isOutput=False)
    sel = nc.declare_dram_parameter("sel", [2, 128, 128], bf16, isOutput=False)
    wkvk = nc.declare_dram_parameter("wkvk", [KVLR, 512], bf16, isOutput=False)
    wkvv = nc.declare_dram_parameter("wkvv", [KVLR, 512], bf16, isOutput=False)
    wo = nc.declare_dram_parameter("wo", [HPC * DV, HID], bf16, isOutput=False)
    cs = nc.declare_dram_parameter("cs", [128, T], bf16, isOutput=False)  # [c;c;-s;s]
    masks = nc.declare_dram_parameter("masks", [4, 128, 512], bf16, isOutput=False)
    eye2 = nc.declare_dram_parameter("eye2", [128, 64], bf16, isOutput=False)
    out = nc.declare_dram_parameter("out", [T, HID], f32, isOutput=True)

    KVR = KVLR + 2 * DR  # 640 rows: kv_latn | krot | krotswap
    ag_in_kv = nc.dram_tensor("ag_in_kv", [KVR, NB], bf16)
    ag_out_kv = nc.dram_tensor("ag_out_kv", [4, KVR, NB], bf16)
    HB = NB // 2
    ag_in_q0 = nc.dram_tensor("ag_in_q0", [QLR, HB], bf16)
    ag_out_q0 = nc.dram_tensor("ag_out_q0", [4, QLR, HB], bf16)
    ag_in_q1 = nc.dram_tensor("ag_in_q1", [QLR, HB], bf16)
    ag_out_q1 = nc.dram_tensor("ag_out_q1", [4, QLR, HB], bf16)

    with tile.TileContext(nc) as tc:
        # ============ phase 1: latents for own 512-token block ============
        with (
            tc.tile_pool(name="p1", bufs=1) as p1,
            tc.tile_pool(name="p1w", bufs=2) as p1w,
            tc.tile_pool(name="ps1", bufs=3, space="PSUM") as ps1,
            tc.tile_pool(name="ps1acc", bufs=1, space="PSUM") as ps1acc,
        ):
            xt = p1.tile([128, 16, NB], bf16, tag="xt")
            wqat = p1.tile([128, 16, QLR], bf16, tag="wqat")
            wkvat = p1.tile([128, 16, KVLR + 2 * DR], bf16, tag="wkvat")
            for k in range(16):
                nc.sync.dma_start(xt[:, k], xT[128 * k : 128 * (k + 1), :])
                nc.sync.dma_start(wkvat[:, k], wkva[128 * k : 128 * (k + 1), :])
            for k in range(16):
                nc.sync.dma_start(wqat[:, k], wqa[128 * k : 128 * (k + 1), :])
            ones = p1.tile([128, 128], bf16, tag="ones")
            nc.vector.memset(ones[:], 1.0)

            def normalize_and_store(ssq, latt, mt, d, dst_, w):
                mt_ = p1w.tile([128, w], f32, tag="rmst", name="rmst")
                nc.vector.tensor_scalar(
                    mt_[:], ssq[:], 1.0 / d, EPS,
                    mybir.AluOpType.mult, mybir.AluOpType.add,
                )
                rms = p1w.tile([128, w], f32, tag="rms", name="rms")
                nc.scalar.activation(rms[:], mt_[:], Sqrt)
                inv = p1w.tile([128, w], f32, tag="inv", name="inv")
                nc.vector.reciprocal(inv[:], rms[:])
                for m in range(mt):
                    ltn = p1w.tile([128, w], bf16, tag="ltn", name="ltn")
                    nc.vector.tensor_mul(ltn[:], latt[:, m], inv[:])
                    nc.gpsimd.dma_start(dst_[128 * m : 128 * (m + 1), :], ltn[:])

            # ckv^T first: m 0..3 kv_lat (normed), m 4 = krot+krotswap (raw)
            kvlat = p1.tile([128, 4, NB], f32, tag="kvlat")
            ssq_kv = ps1acc.tile([128, NB], f32, tag="ssq_kv")
            for m in range(5):
                ps = ps1.tile([128, NB], f32, tag="p1ps")
                for k in range(16):
                    nc.tensor.matmul(
                        ps[:], wkvat[:, k, 128 * m : 128 * (m + 1)], xt[:, k],
                        start=(k == 0), stop=(k == 15),
                    )
                if m < 4:
                    sq = p1w.tile([128, NB], bf16, tag="sq")
                    nc.scalar.activation(sq[:], ps[:], Square)
                    nc.vector.tensor_copy(kvlat[:, m], ps[:])
                    nc.tensor.matmul(ssq_kv[:], ones[:], sq[:],
                                     start=(m == 0), stop=(m == 3))
                else:
                    rot = p1w.tile([128, NB], bf16, tag="rot")
                    nc.vector.tensor_copy(rot[:], ps[:])
                    nc.gpsimd.dma_start(ag_in_kv[KVLR : KVR, :], rot[:])
            normalize_and_store(ssq_kv, kvlat, 4, KVLR, ag_in_kv, NB)

            nc.gpsimd.collective_compute(
                "AllGather", mybir.AluOpType.bypass,
                replica_groups=[[0, 1, 2, 3], [4, 5, 6, 7]],
                ins=[ag_in_kv[:]], outs=[ag_out_kv[:]],
            )

            # q_lat^T in two column halves so AG_q0 can start early
            for half, (agi, ago_) in enumerate(
                ((ag_in_q0, ag_out_q0), (ag_in_q1, ag_out_q1))
            ):
                c0 = half * HB
                qlat = p1.tile([128, 12, HB], f32, tag="qlat", name="qlat")
                ssq_q = ps1acc.tile([128, HB], f32, tag="ssq_q", name="ssq_q")
                for m in range(12):
                    ps = ps1.tile([128, HB], f32, tag="p1psq", name="p1psq")
                    for k in range(16):
                        nc.tensor.matmul(
                            ps[:], wqat[:, k, 128 * m : 128 * (m + 1)],
                            xt[:, k, c0 : c0 + HB],
                            start=(k == 0), stop=(k == 15),
                        )
                    sq = p1w.tile([128, HB], bf16, tag="sqq", name="sqq")
                    nc.scalar.activation(sq[:], ps[:], Square)
                    nc.vector.tensor_copy(qlat[:, m], ps[:])
                    nc.tensor.matmul(ssq_q[:], ones[:], sq[:],
                                     start=(m == 0), stop=(m == 11))
                normalize_and_store(ssq_q, qlat, 12, QLR, agi, HB)
                nc.gpsimd.collective_compute(
                    "AllGather", mybir.AluOpType.bypass,
                    replica_groups=[[0, 1, 2, 3], [4, 5, 6, 7]],
                    ins=[agi[:]], outs=[ago_[:]],
                )

        # ============ phase 2: 4 heads, full batch ============
        with tc.tile_pool(name="p2", bufs=1) as p2:
            qTp = p2.tile([128, 4, 4, NB], bf16, tag="qTp")      # pass, per head m
            qrot = [
                p2.tile([64, T], bf16, tag=f"qrot{h}", name=f"qrot{h}")
                for h in range(HPC)
            ]
            krotT = p2.tile([64, T], bf16, tag="krotT")
            kpT = p2.tile([128, 4, 4, NB], bf16, tag="kpT")      # [d, head, r, t]
            vT = p2.tile([128, 16, 512], bf16, tag="vT")         # [t, t-tile, dv]
            attnT = p2.tile([128, 4, 4, NB], bf16, tag="attnT")  # [dv, head, qn, t]
            cst = p2.tile([128, T], bf16, tag="cst")
            nc.sync.dma_start(cst[:], cs[:])
            eyet = p2.tile([128, 64], bf16, tag="eyet")
            nc.sync.dma_start(eyet[:], eye2[:])
            maskt = p2.tile([128, 4, 512], bf16, tag="maskt")
            for m in range(4):
                nc.sync.dma_start(maskt[:, m], masks[m])
            onesb = p2.tile([128, 128], bf16, tag="onesb")
            nc.vector.memset(onesb[:], 1.0)

            # ---- 2b/2c: k_pass^T, V, k_rot rope ----
            with (
                tc.tile_pool(name="p2b", bufs=1) as p2b,
                tc.tile_pool(name="ps2", bufs=2, space="PSUM") as ps2,
            ):
                wkkt = p2b.tile([128, 4, 512], bf16, tag="wkkt")
                wkvt = p2b.tile([128, 4, 512], bf16, tag="wkvt")
                for k in range(4):
                    nc.sync.dma_start(wkkt[:, k], wkvk[128 * k : 128 * (k + 1), :])
                    nc.sync.dma_start(wkvt[:, k], wkvv[128 * k : 128 * (k + 1), :])
                kvl = p2b.tile([128, 4, 4, NB], bf16, tag="kvl")  # [r_lat, k, r, t]
                krr = p2b.tile([128, 4, NB], bf16, tag="krr")
                for r in range(4):
                    for k in range(4):
                        nc.scalar.dma_start(
                            kvl[:, k, r],
                            ag_out_kv[r, 128 * k : 128 * (k + 1), :],
                        )
                    nc.scalar.dma_start(krr[:, r], ag_out_kv[r, KVLR : KVR, :])
                # k_pass^T
                for m in range(4):
                    for r in range(4):
                        ps = ps2.tile([128, NB], f32, tag="k2ps")
                        for k in range(4):
                            nc.tensor.matmul(
                                ps[:], wkkt[:, k, 128 * m : 128 * (m + 1)],
                                kvl[:, k, r], start=(k == 0), stop=(k == 3),
                            )
                        nc.vector.tensor_copy(kpT[:, m, r], ps[:])
                # V token-major
                for r in range(4):
                    for s in range(4):
                        ps = ps2.tile([128, 512], f32, tag="v2ps")
                        for k in range(4):
                            nc.tensor.matmul(
                                ps[:], kvl[:, k, r, 128 * s : 128 * (s + 1)],
                                wkvt[:, k], start=(k == 0), stop=(k == 3),
                            )
                        nc.vector.tensor_copy(vT[:, 4 * r + s], ps[:])
                # k_rot rope
                for r in range(4):
                    tt = p2b.tile([128, NB], bf16, tag="kropet")
                    nc.vector.tensor_mul(
                        tt[:], krr[:, r], cst[:, 512 * r : 512 * (r + 1)]
                    )
                    pr = ps2.tile([64, NB], f32, tag="kropeps")
                    nc.tensor.matmul(pr[:], eyet[:], tt[:], start=True, stop=True)
                    nc.vector.tensor_copy(krotT[:, 512 * r : 512 * (r + 1)], pr[:])

            # ---- 2a: q^T (8 m-tiles x 4 blocks, K=12) + rope ----
            with (
                tc.tile_pool(name="p2a", bufs=1) as p2a,
                tc.tile_pool(name="p2as", bufs=2) as p2as,
                tc.tile_pool(name="ps2", bufs=2, space="PSUM") as ps2,
            ):
                wqbt = p2a.tile([128, 12, 768], bf16, tag="wqbt")
                selt = p2a.tile([128, 2, 128], bf16, tag="selt")
                for v in range(2):
                    nc.sync.dma_start(selt[:, v], sel[v])
                for k in range(12):
                    nc.sync.dma_start(wqbt[:, k], wqb[128 * k : 128 * (k + 1), :])
                qrw = p2a.tile([128, 2, 4, NB], bf16, tag="qrw")  # raw rot pairs
                for c, agq in ((0, ag_out_q0), (1, ag_out_q1)):
                    for r in range(4):
                        qlt = p2as.tile([128, 12, HB], bf16, tag="qlt")
                        for k in range(12):
                            nc.scalar.dma_start(
                                qlt[:, k], agq[r, 128 * k : 128 * (k + 1), :]
                            )
                        for m in range(6):
                            ps = ps2.tile([128, HB], f32, tag="q2ps")
                            for k in range(12):
                                nc.tensor.matmul(
                                    ps[:], wqbt[:, k, 128 * m : 128 * (m + 1)],
                                    qlt[:, k], start=(k == 0), stop=(k == 11),
                                )
                            dst = qTp[:, m, r] if m < 4 else qrw[:, m - 4, r]
                            nc.vector.tensor_copy(
                                dst[:, c * HB : (c + 1) * HB], ps[:]
                            )
                # rope q: sel-matmul builds [rot_h; rotswap_h], then
                # CS-mul (from PSUM) + identity-stack matmul
                for h in range(HPC):
                    for r in range(4):
                        sp = ps2.tile([128, NB], f32, tag="selps", name="selps")
                        nc.tensor.matmul(sp[:], selt[:, h % 2],
                                         qrw[:, h // 2, r], start=True, stop=True)
                        tt = p2as.tile([128, NB], bf16, tag="ropet")
                        nc.vector.tensor_mul(
                            tt[:], sp[:], cst[:, 512 * r : 512 * (r + 1)]
                        )
                        pr = ps2.tile([64, NB], f32, tag="ropeps")
                        nc.tensor.matmul(pr[:], eyet[:], tt[:], start=True, stop=True)
                        nc.vector.tensor_copy(
                            qrot[h][:, 512 * r : 512 * (r + 1)], pr[:]
                        )

            wot = p2.tile([128, 4, HID], bf16, tag="wot")
            for k in range(4):
                nc.sync.dma_start(wot[:, k], wo[128 * k : 128 * (k + 1), :])

            # ---- 2d: attention ----
            with (
                tc.tile_pool(name="p2d", bufs=6) as p2d,
                tc.tile_pool(name="p2dn", bufs=2) as p2dn,
                tc.tile_pool(name="ps2s", bufs=4, space="PSUM") as ps2s,
                tc.tile_pool(name="ps2acc", bufs=2, space="PSUM") as ps2acc,
            ):
                for h in range(HPC):
                    for qn in range(4):
                        nkt = 4 * qn + 4
                        aps = ps2acc.tile([128, NB], f32, tag="attn_ps")
                        sps = ps2acc.tile([128, NB], f32, tag="sum_ps")
                        eac = p2dn.tile([128, NB], bf16, tag="eacc", name="eacc")
                        prev = None

                        def emit_pv(pe, pk, off, last):
                            nc.tensor.matmul(
                                aps[:, off:NB], vT[:, pk, 128 * h : 128 * (h + 1)],
                                pe[:, off:NB], start=(pk == 0), stop=last,
                            )
                            if pk == 0:
                                nc.vector.tensor_copy(eac[:], pe[:])
                            else:
                                nc.vector.tensor_add(
                                    eac[:, off:NB], eac[:, off:NB], pe[:, off:NB]
                                )
                            if last:
                                nc.tensor.matmul(
                                    sps[:], onesb[:], eac[:],
                                    start=True, stop=True,
                                )

                        for kt in range(nkt):
                            m = kt - 4 * qn
                            off = 128 * m if m > 0 else 0
                            scp = ps2s.tile([128, NB], f32, tag="scps")
                            r, sl = kt // 4, 128 * (kt % 4)
                            nc.tensor.matmul(
                                scp[:, off:NB], kpT[:, h, r, sl : sl + 128],
                                qTp[:, h, qn, off:NB], start=True, stop=False,
                            )
                            nc.tensor.matmul(
                                scp[:, off:NB], krotT[:, 128 * kt : 128 * kt + 128],
                                qrot[h][:, 512 * qn + off : 512 * (qn + 1)],
                                start=False, stop=True,
                            )
                            et = p2d.tile([128, NB], bf16, tag="expT")
                            nc.scalar.activation(
                                et[:, off:NB], scp[:, off:NB], Exp, scale=SCALE
                            )
                            if m >= 0:
                                nc.vector.tensor_mul(
                                    et[:, off:NB], et[:, off:NB],
                                    maskt[:, m, off:NB],
                                )
                            if prev is not None:
                                emit_pv(prev[0], prev[1], prev[2], False)
                            prev = (et, kt, off)
                        emit_pv(prev[0], prev[1], prev[2], True)
                        rec = p2dn.tile([128, NB], f32, tag="rec")
                        nc.vector.reciprocal(rec[:], sps[:])
                        nc.vector.tensor_mul(attnT[:, h, qn], aps[:], rec[:])

            # ---- 2e: o_proj partial (token-major out) ----
            with (
                tc.tile_pool(name="p2eo", bufs=3) as p2eo,
                tc.tile_pool(name="ps2o", bufs=3, space="PSUM") as ps2o,
            ):
                for tm in range(16):
                    qn, s = tm // 4, 128 * (tm % 4)
                    for n in range(4):
                        ps = ps2o.tile([128, 512], f32, tag="ops")
                        for k in range(4):
                            nc.tensor.matmul(
                                ps[:], attnT[:, k, qn, s : s + 128],
                                wot[:, k, 512 * n : 512 * (n + 1)],
                                start=(k == 0), stop=(k == 3),
                            )
                        ot = p2eo.tile([128, 512], f32, tag="oT")
                        nc.vector.tensor_copy(ot[:], ps[:])
                        nc.sync.dma_start(
                            out[128 * tm : 128 * (tm + 1),
                                512 * n : 512 * (n + 1)], ot[:],
                        )

    nc.finalize()
    return nc


_NC = None


def _get_nc():
    global _NC
    if _NC is None:
        _NC = _build()
    return _NC


def _prep_inputs(x, attention_mask, positions, wqa, qa_scale, wqb, wkva,
                 kva_scale, wkvb, wo):
    x = np.asarray(x, np.float32)
    positions = np.asarray(positions)
    wqa = np.asarray(wqa, np.float32)
    wqb = np.asarray(wqb, np.float32) * np.asarray(qa_scale, np.float32)[:, None]
    wkva = np.asarray(wkva, np.float32)
    wkvb = np.asarray(wkvb, np.float32) * np.asarray(kva_scale, np.float32)[:, None]
    wo = np.asarray(wo, np.float32)

    # wkva augmented with swapped-rot columns
    kr = wkva[:, KVLR:]
    wkva_aug = np.concatenate(
        [wkva[:, :KVLR], kr, kr[:, DR // 2 :], kr[:, : DR // 2]], axis=1
    ).astype(_BF)

    # masks: mask[m][r, c] = c >= 128*m + r
    rr = np.arange(128)[:, None]
    cc = np.arange(512)[None, :]
    masks = np.stack([(cc >= 128 * m + rr) for m in range(4)]).astype(_BF)

    eye2 = np.concatenate([np.eye(64), np.eye(64)], axis=0).astype(_BF)

    # sel[v]: out rows [0:64]=src rows [64v:64v+64]; [64:128]=32-swapped copy
    sel = np.zeros((2, 128, 128), np.float32)
    for v in range(2):
        for i in range(64):
            sel[v, 64 * v + i, i] = 1.0
            sel[v, 64 * v + ((i + 32) % 64), 64 + i] = 1.0
    sel = sel.astype(_BF)

    # per-batch cos/sin stack [c; c; -s; s]
    inv_freq = 1.0 / (THETA ** (np.arange(0, DR, 2, dtype=np.float32) / DR))
    cs_b = []
    for b in range(B):
        ang = positions[b].astype(np.float32)[None, :] * inv_freq[:, None]
        c, s = np.cos(ang), np.sin(ang)
        cs_b.append(np.concatenate([c, c, -s, s], axis=0).astype(_BF))

    wqa_bf = wqa.astype(_BF)
    in_maps = []
    for core in range(8):
        b, j = core // 4, core % 4
        hs = [4 * (core % 4) + i for i in range(HPC)]
        # wqb per head-group: [pass x4 | (rot,swap) per head x4]
        cols = [wqb[:, h * DQK : h * DQK + DN] for h in hs]
        for h in hs:
            cols.append(wqb[:, h * DQK + DN : (h + 1) * DQK])
        wqb_hg = np.concatenate(cols, axis=1).astype(_BF)
        wkvk_hg = np.concatenate(
            [wkvb[:, h * (DN + DV) : h * (DN + DV) + DN] for h in hs], axis=1
        ).astype(_BF)
        wkvv_hg = np.concatenate(
            [wkvb[:, h * (DN + DV) + DN : (h + 1) * (DN + DV)] for h in hs], axis=1
        ).astype(_BF)
        wo_hg = wo[hs[0] * DV : (hs[-1] + 1) * DV, :].astype(_BF)
        xTb = np.ascontiguousarray(
            x[b, NB * j : NB * (j + 1), :].T).astype(_BF)
        in_maps.append({
            "xT": xTb,
            "wqa": wqa_bf,
            "wkva": wkva_aug,
            "wqb": wqb_hg,
            "wkvk": wkvk_hg,
            "wkvv": wkvv_hg,
            "wo": wo_hg,
            "cs": cs_b[b],
            "masks": masks,
            "eye2": eye2,
            "sel": sel,
        })
    return in_maps


def _run(inputs, trace=False, trace_kwargs=None):
    from concourse.bass_utils import run_bass_kernel_spmd

    nc = _get_nc()
    in_maps = _prep_inputs(**inputs)
    res = run_bass_kernel_spmd(
        nc, in_maps, list(range(8)), trace=trace,
        trace_kwargs=trace_kwargs or {},
    )
    outs = np.zeros((B, T, HID), np.float32)
    for core in range(8):
        outs[core // 4] += res.results[core]["out"]
    return outs, res


def kernel(**inputs) -> np.ndarray:
    out, _ = _run(inputs)
    return out
